# revision 5
# baseline (speedup 1.0000x reference)
"""KAN encoder (2 KAN layers + relu + linear head) on 8 trn2 NeuronCores.

Strategy: data-parallel on batch (512 rows/core), all weights replicated.
Layout is feature-on-partition / batch-on-free throughout, so no device-side
transposes are needed. The spline path is a dense matmul over (in*8) with the
B-spline bases computed on the ACT+DVE engines via the exact identity

    bases_k(x) * 6 = relu(2 - |u-k-2|)^3 - 4*relu(1 - |u-k-2|)^3,
    u = (x + 2.2) / 0.4

(the 1/6 is folded into the spline weights host-side, as is the spline_scaler).
All matmuls run in float32r (tf32-class precision, 1 cycle/row at N=512).
"""
import numpy as np
from contextlib import ExitStack

from concourse import bacc, tile, mybir
from concourse.bass_utils import run_bass_kernel_spmd

F32 = mybir.dt.float32
F32R = mybir.dt.float32r
AF = mybir.ActivationFunctionType

B, D_IN, H0, H1, L = 4096, 1024, 2048, 1024, 512
NCORES = 8
BC = B // NCORES          # 512 batch cols per core
NB = 512                  # free-dim (batch) tile = full per-core batch
CBRT4 = float(4.0 ** (1.0 / 3.0))

_cache = {}


def _build_full():
    nc = bacc.Bacc("TRN2", target_bir_lowering=False, debug=False,
                   num_devices=NCORES)

    x_d = nc.dram_tensor("x_d", [8, 128, BC], F32, kind="ExternalInput")
    w0_d = nc.dram_tensor("w0_d", [8, 128, 9, H0], F32R, kind="ExternalInput")
    w1_d = nc.dram_tensor("w1_d", [16, 128, 9, H1], F32R, kind="ExternalInput")
    dw_d = nc.dram_tensor("dw_d", [8, 128, L], F32R, kind="ExternalInput")
    db_d = nc.dram_tensor("db_d", [128, 4], F32, kind="ExternalInput")
    o_d = nc.dram_tensor("o_d", [4, 128, BC], F32, kind="ExternalOutput")

    with tile.TileContext(nc) as tc, ExitStack() as ctx:
        cpool = ctx.enter_context(tc.tile_pool(name="cpool", bufs=1))
        psum = ctx.enter_context(tc.tile_pool(name="psum", bufs=1, space="PSUM"))
        h0p = ctx.enter_context(tc.tile_pool(name="h0p", bufs=1))
        ab = ctx.enter_context(tc.tile_pool(name="ab", bufs=1))

        bias_tiles = {}

        def bias_ap(val):
            val = float(val)
            if val not in bias_tiles:
                t = cpool.tile([128, 1], F32, tag=f"b{len(bias_tiles)}",
                               name=f"bias{len(bias_tiles)}")
                nc.gpsimd.memset(t[:, :], val)
                bias_tiles[val] = t
            return bias_tiles[val][:, :]

        h0_sb = h0p.tile([128, 16 * NB], F32, name="h0_sb")

        def emit_feats(src_ap, feats, tagp):
            """feats (128, 9*NB) f32: j=0 silu(src); j=1+k -> bases_k(src)*6."""
            nc.scalar.activation(feats[:, 0:NB], src_ap, AF.Silu,
                                 bias=bias_ap(0.0), scale=1.0)
            for k in range(8):
                s = ab.tile([128, NB], F32, tag="s", bufs=2, name=f"s{tagp}_{k}")
                a = ab.tile([128, NB], F32, tag="a", bufs=2, name=f"a{tagp}_{k}")
                e = ab.tile([128, NB], F32, tag="e", bufs=2, name=f"e{tagp}_{k}")
                a2 = ab.tile([128, NB], F32, tag="a2", bufs=2, name=f"a2{tagp}_{k}")
                a3 = ab.tile([128, NB], F32, tag="a3", bufs=2, name=f"a3{tagp}_{k}")
                e2 = ab.tile([128, NB], F32, tag="e2", bufs=2, name=f"e2{tagp}_{k}")
                e3 = ab.tile([128, NB], F32, tag="e3", bufs=2, name=f"e3{tagp}_{k}")
                nc.scalar.activation(s[:, :], src_ap, AF.Abs,
                                     bias=bias_ap(3.5 - k), scale=2.5)
                nc.scalar.activation(a[:, :], s[:, :], AF.Relu,
                                     bias=bias_ap(2.0), scale=-1.0)
                nc.scalar.activation(e[:, :], s[:, :], AF.Relu,
                                     bias=bias_ap(CBRT4), scale=-CBRT4)
                nc.vector.tensor_mul(a2[:, :], a[:, :], a[:, :])
                nc.vector.tensor_mul(a3[:, :], a2[:, :], a[:, :])
                nc.gpsimd.tensor_mul(e2[:, :], e[:, :], e[:, :])
                nc.gpsimd.tensor_mul(e3[:, :], e2[:, :], e[:, :])
                nc.vector.tensor_sub(feats[:, (k + 1) * NB:(k + 2) * NB],
                                     a3[:, :], e3[:, :])

        # ---- Layer 0: out chunks in two groups of 8 PSUM banks ----
        for og in range(2):
            pts = [psum.tile([128, NB], F32, tag=f"bank{oc}",
                             name=f"psA{og}_{oc}") for oc in range(8)]
            for ic in range(8):
                xt = ab.tile([128, NB], F32, tag="xt", bufs=2, name=f"xt{og}_{ic}")
                nc.sync.dma_start(out=xt[:, :], in_=x_d[ic, :, :])
                feats = ab.tile([128, 9 * NB], F32R, tag="feats", bufs=2,
                                name=f"f0_{og}_{ic}")
                emit_feats(xt[:, :], feats, f"0_{og}_{ic}")
                wsl = ab.tile([128, 9, 8, 128], F32R, tag="wsl", bufs=2,
                              name=f"w0_{og}_{ic}")
                nc.sync.dma_start(out=wsl[:, :, :, :],
                                  in_=w0_d[ic, :, :, og * 1024:(og + 1) * 1024])
                for oc in range(8):
                    for j in range(9):
                        nc.tensor.matmul(
                            pts[oc][:, :], wsl[:, j, oc, :],
                            feats[:, j * NB:(j + 1) * NB],
                            start=(ic == 0 and j == 0),
                            stop=(ic == 7 and j == 8))
            for oc in range(8):
                nc.scalar.activation(
                    h0_sb[:, (og * 8 + oc) * NB:(og * 8 + oc + 1) * NB],
                    pts[oc][:, :], AF.Copy, bias=0.0, scale=1.0)

        # ---- Layer 1: 8 out chunks, 16 contraction chunks ----
        pts = [psum.tile([128, NB], F32, tag=f"bank{oc}", name=f"psB{oc}")
               for oc in range(8)]
        for ic in range(16):
            feats = ab.tile([128, 9 * NB], F32R, tag="feats", bufs=2,
                            name=f"f1_{ic}")
            emit_feats(h0_sb[:, ic * NB:(ic + 1) * NB], feats, f"1_{ic}")
            wsl = ab.tile([128, 9, 8, 128], F32R, tag="wsl", bufs=2,
                          name=f"w1_{ic}")
            nc.sync.dma_start(out=wsl[:, :, :, :], in_=w1_d[ic, :, :, :])
            for oc in range(8):
                for j in range(9):
                    nc.tensor.matmul(
                        pts[oc][:, :], wsl[:, j, oc, :],
                        feats[:, j * NB:(j + 1) * NB],
                        start=(ic == 0 and j == 0),
                        stop=(ic == 15 and j == 8))

        # ---- Head: relu(h1) @ dw.T + db ----
        rl = ab.tile([128, 8 * NB], F32R, tag="feats", bufs=2, name="rl")
        for oc in range(8):
            nc.scalar.activation(rl[:, oc * NB:(oc + 1) * NB],
                                 pts[oc][:, :], AF.Relu,
                                 bias=bias_ap(0.0), scale=1.0)
        dwt = ab.tile([128, 8, L], F32R, tag="wsl", bufs=2, name="dwt")
        for ic in range(8):
            nc.sync.dma_start(out=dwt[:, ic, :], in_=dw_d[ic, :, :])
        dbt = cpool.tile([128, 4], F32, name="dbt")
        nc.sync.dma_start(out=dbt[:, :], in_=db_d[:, :])
        out_sb = ab.tile([128, 4 * NB], F32, tag="outsb", name="out_sb")
        for lc in range(4):
            pt = psum.tile([128, NB], F32, tag=f"bank{lc}", name=f"psC{lc}")
            for ic in range(8):
                nc.tensor.matmul(pt[:, :], dwt[:, ic, lc * 128:(lc + 1) * 128],
                                 rl[:, ic * NB:(ic + 1) * NB],
                                 start=(ic == 0), stop=(ic == 7))
            nc.scalar.activation(out_sb[:, lc * NB:(lc + 1) * NB], pt[:, :],
                                 AF.Identity, bias=dbt[:, lc:lc + 1], scale=1.0)
            nc.sync.dma_start(out=o_d[lc, :, :],
                              in_=out_sb[:, lc * NB:(lc + 1) * NB])

    nc.compile()
    return nc


def _prep_weights(bw0, sw0, ss0, bw1, sw1, ss1, dw, db):
    # layer0: (8 ic, 128 r, 9 j, 2048 o); j=0 -> bw0.T, j=1+k -> (sw0*ss0/6).T
    w0 = np.empty((8, 128, 9, H0), np.float32)
    w0[:, :, 0, :] = bw0.T.reshape(8, 128, H0)
    s0 = (sw0 * (ss0[:, :, None] / 6.0)).transpose(1, 2, 0)  # (i, k, o)
    w0[:, :, 1:, :] = s0.reshape(8, 128, 8, H0)
    w1 = np.empty((16, 128, 9, H1), np.float32)
    w1[:, :, 0, :] = bw1.T.reshape(16, 128, H1)
    s1 = (sw1 * (ss1[:, :, None] / 6.0)).transpose(1, 2, 0)
    w1[:, :, 1:, :] = s1.reshape(16, 128, 8, H1)
    dwt = np.ascontiguousarray(dw.T.reshape(8, 128, L))
    dbt = np.ascontiguousarray(db.reshape(4, 128).T)
    return (np.ascontiguousarray(w0), np.ascontiguousarray(w1), dwt, dbt)


def kernel(x, bw0, sw0, ss0, bw1, sw1, ss1, dw, db):
    if "nc" not in _cache:
        _cache["nc"] = _build_full()
    nc = _cache["nc"]
    w0, w1, dwt, dbt = _prep_weights(
        np.asarray(bw0, np.float32), np.asarray(sw0, np.float32),
        np.asarray(ss0, np.float32), np.asarray(bw1, np.float32),
        np.asarray(sw1, np.float32), np.asarray(ss1, np.float32),
        np.asarray(dw, np.float32), np.asarray(db, np.float32))
    xT = np.ascontiguousarray(np.asarray(x, np.float32).T)  # (1024, 4096)
    in_maps = []
    for c in range(NCORES):
        xc = np.ascontiguousarray(
            xT[:, c * BC:(c + 1) * BC].reshape(8, 128, BC))
        in_maps.append({"x_d": xc, "w0_d": w0, "w1_d": w1,
                        "dw_d": dwt, "db_d": dbt})
    _cache["in_maps"] = in_maps
    res = run_bass_kernel_spmd(nc, in_maps, list(range(NCORES)))
    out = np.empty((B, L), np.float32)
    for c in range(NCORES):
        oc = res.results[c]["o_d"]          # (4, 128, BC)
        out[c * BC:(c + 1) * BC, :] = oc.reshape(L, BC).T
    return out


# revision 6
# speedup vs baseline: 1.4173x; 1.4173x over previous
"""KAN encoder (2 KAN layers + relu + linear head) on 8 trn2 NeuronCores.

Strategy: data-parallel on batch (512 rows/core), all weights replicated.
Layout is feature-on-partition / batch-on-free throughout, so no device-side
transposes are needed. The spline path is a dense matmul over (in*8) with the
B-spline bases computed on the ACT+DVE engines via the exact identity

    bases_k(x) * 6 = relu(2 - |u-k-2|)^3 - 4*relu(1 - |u-k-2|)^3,
    u = (x + 2.2) / 0.4

(the 1/6 is folded into the spline weights host-side, as is the spline_scaler).
All matmuls run in float32r (tf32-class precision, 1 cycle/row at N=512).
"""
import numpy as np
from contextlib import ExitStack

from concourse import bacc, tile, mybir
from concourse.bass_utils import run_bass_kernel_spmd

F32 = mybir.dt.float32
F32R = mybir.dt.float32r
AF = mybir.ActivationFunctionType

B, D_IN, H0, H1, L = 4096, 1024, 2048, 1024, 512
NCORES = 8
BC = B // NCORES          # 512 batch cols per core
NB = 512                  # free-dim (batch) tile = full per-core batch
CBRT4 = float(4.0 ** (1.0 / 3.0))

_cache = {}


def _build_full():
    nc = bacc.Bacc("TRN2", target_bir_lowering=False, debug=False,
                   num_devices=NCORES)

    x_d = nc.dram_tensor("x_d", [8, 128, BC], F32, kind="ExternalInput")
    w0_d = nc.dram_tensor("w0_d", [8, 128, 9, H0], F32R, kind="ExternalInput")
    w1_d = nc.dram_tensor("w1_d", [16, 128, 9, H1], F32R, kind="ExternalInput")
    dw_d = nc.dram_tensor("dw_d", [8, 128, L], F32R, kind="ExternalInput")
    db_d = nc.dram_tensor("db_d", [128, 4], F32, kind="ExternalInput")
    o_d = nc.dram_tensor("o_d", [4, 128, BC], F32, kind="ExternalOutput")

    with tile.TileContext(nc) as tc, ExitStack() as ctx:
        cpool = ctx.enter_context(tc.tile_pool(name="cpool", bufs=1))
        psum = ctx.enter_context(tc.tile_pool(name="psum", bufs=1, space="PSUM"))
        h0p = ctx.enter_context(tc.tile_pool(name="h0p", bufs=1))
        ab = ctx.enter_context(tc.tile_pool(name="ab", bufs=1))

        bias_tiles = {}

        def bias_ap(val):
            val = float(val)
            if val not in bias_tiles:
                t = cpool.tile([128, 1], F32, tag=f"b{len(bias_tiles)}",
                               name=f"bias{len(bias_tiles)}")
                nc.gpsimd.memset(t[:, :], val)
                bias_tiles[val] = t
            return bias_tiles[val][:, :]

        h0_sb = h0p.tile([128, 16 * NB], F32, name="h0_sb")

        def emit_feats(src_ap, feats, tagp):
            """feats (128, 9*NB) f32: j=0 silu(src); j=1+k -> bases_k(src)*6."""
            nc.scalar.activation(feats[:, 0:NB], src_ap, AF.Silu,
                                 bias=bias_ap(0.0), scale=1.0)
            for k in range(8):
                s = ab.tile([128, NB], F32, tag="s", bufs=2, name=f"s{tagp}_{k}")
                a = ab.tile([128, NB], F32, tag="a", bufs=2, name=f"a{tagp}_{k}")
                e = ab.tile([128, NB], F32, tag="e", bufs=2, name=f"e{tagp}_{k}")
                a2 = ab.tile([128, NB], F32, tag="a2", bufs=2, name=f"a2{tagp}_{k}")
                a3 = ab.tile([128, NB], F32, tag="a3", bufs=2, name=f"a3{tagp}_{k}")
                e2 = ab.tile([128, NB], F32, tag="e2", bufs=2, name=f"e2{tagp}_{k}")
                e3 = ab.tile([128, NB], F32, tag="e3", bufs=2, name=f"e3{tagp}_{k}")
                nc.scalar.activation(s[:, :], src_ap, AF.Abs,
                                     bias=bias_ap(3.5 - k), scale=2.5)
                nc.scalar.activation(a[:, :], s[:, :], AF.Relu,
                                     bias=bias_ap(2.0), scale=-1.0)
                nc.scalar.activation(e[:, :], s[:, :], AF.Relu,
                                     bias=bias_ap(CBRT4), scale=-CBRT4)
                nc.vector.tensor_mul(a2[:, :], a[:, :], a[:, :])
                nc.vector.tensor_mul(a3[:, :], a2[:, :], a[:, :])
                if k % 3 != 0:
                    nc.scalar.activation(e2[:, :], e[:, :], AF.Square,
                                         bias=bias_ap(0.0), scale=1.0)
                else:
                    nc.vector.tensor_mul(e2[:, :], e[:, :], e[:, :])
                nc.vector.tensor_mul(e3[:, :], e2[:, :], e[:, :])
                nc.vector.tensor_sub(feats[:, (k + 1) * NB:(k + 2) * NB],
                                     a3[:, :], e3[:, :])

        # ---- Layer 0: out chunks in two groups of 8 PSUM banks ----
        for og in range(2):
            pts = [psum.tile([128, NB], F32, tag=f"bank{oc}",
                             name=f"psA{og}_{oc}") for oc in range(8)]
            for ic in range(8):
                xt = ab.tile([128, NB], F32, tag="xt", bufs=2, name=f"xt{og}_{ic}")
                nc.sync.dma_start(out=xt[:, :], in_=x_d[ic, :, :])
                feats = ab.tile([128, 9 * NB], F32R, tag="feats", bufs=2,
                                name=f"f0_{og}_{ic}")
                emit_feats(xt[:, :], feats, f"0_{og}_{ic}")
                wsl = ab.tile([128, 9, 8, 128], F32R, tag="wsl", bufs=2,
                              name=f"w0_{og}_{ic}")
                nc.sync.dma_start(out=wsl[:, :, :, :],
                                  in_=w0_d[ic, :, :, og * 1024:(og + 1) * 1024])
                for oc in range(8):
                    for j in range(9):
                        nc.tensor.matmul(
                            pts[oc][:, :], wsl[:, j, oc, :],
                            feats[:, j * NB:(j + 1) * NB],
                            start=(ic == 0 and j == 0),
                            stop=(ic == 7 and j == 8))
            for oc in range(8):
                nc.scalar.activation(
                    h0_sb[:, (og * 8 + oc) * NB:(og * 8 + oc + 1) * NB],
                    pts[oc][:, :], AF.Copy, bias=0.0, scale=1.0)

        # ---- Layer 1: 8 out chunks, 16 contraction chunks ----
        pts = [psum.tile([128, NB], F32, tag=f"bank{oc}", name=f"psB{oc}")
               for oc in range(8)]
        for ic in range(16):
            feats = ab.tile([128, 9 * NB], F32R, tag="feats", bufs=2,
                            name=f"f1_{ic}")
            emit_feats(h0_sb[:, ic * NB:(ic + 1) * NB], feats, f"1_{ic}")
            wsl = ab.tile([128, 9, 8, 128], F32R, tag="wsl", bufs=2,
                          name=f"w1_{ic}")
            nc.sync.dma_start(out=wsl[:, :, :, :], in_=w1_d[ic, :, :, :])
            for oc in range(8):
                for j in range(9):
                    nc.tensor.matmul(
                        pts[oc][:, :], wsl[:, j, oc, :],
                        feats[:, j * NB:(j + 1) * NB],
                        start=(ic == 0 and j == 0),
                        stop=(ic == 15 and j == 8))

        # ---- Head: relu(h1) @ dw.T + db ----
        rl = ab.tile([128, 8 * NB], F32R, tag="feats", bufs=2, name="rl")
        for oc in range(8):
            nc.scalar.activation(rl[:, oc * NB:(oc + 1) * NB],
                                 pts[oc][:, :], AF.Relu,
                                 bias=bias_ap(0.0), scale=1.0)
        dwt = ab.tile([128, 8, L], F32R, tag="wsl", bufs=2, name="dwt")
        for ic in range(8):
            nc.sync.dma_start(out=dwt[:, ic, :], in_=dw_d[ic, :, :])
        dbt = cpool.tile([128, 4], F32, name="dbt")
        nc.sync.dma_start(out=dbt[:, :], in_=db_d[:, :])
        out_sb = ab.tile([128, 4 * NB], F32, tag="outsb", name="out_sb")
        for lc in range(4):
            pt = psum.tile([128, NB], F32, tag=f"bank{lc}", name=f"psC{lc}")
            for ic in range(8):
                nc.tensor.matmul(pt[:, :], dwt[:, ic, lc * 128:(lc + 1) * 128],
                                 rl[:, ic * NB:(ic + 1) * NB],
                                 start=(ic == 0), stop=(ic == 7))
            nc.scalar.activation(out_sb[:, lc * NB:(lc + 1) * NB], pt[:, :],
                                 AF.Identity, bias=dbt[:, lc:lc + 1], scale=1.0)
            nc.sync.dma_start(out=o_d[lc, :, :],
                              in_=out_sb[:, lc * NB:(lc + 1) * NB])

    nc.compile()
    return nc


def _prep_weights(bw0, sw0, ss0, bw1, sw1, ss1, dw, db):
    # layer0: (8 ic, 128 r, 9 j, 2048 o); j=0 -> bw0.T, j=1+k -> (sw0*ss0/6).T
    w0 = np.empty((8, 128, 9, H0), np.float32)
    w0[:, :, 0, :] = bw0.T.reshape(8, 128, H0)
    s0 = (sw0 * (ss0[:, :, None] / 6.0)).transpose(1, 2, 0)  # (i, k, o)
    w0[:, :, 1:, :] = s0.reshape(8, 128, 8, H0)
    w1 = np.empty((16, 128, 9, H1), np.float32)
    w1[:, :, 0, :] = bw1.T.reshape(16, 128, H1)
    s1 = (sw1 * (ss1[:, :, None] / 6.0)).transpose(1, 2, 0)
    w1[:, :, 1:, :] = s1.reshape(16, 128, 8, H1)
    dwt = np.ascontiguousarray(dw.T.reshape(8, 128, L))
    dbt = np.ascontiguousarray(db.reshape(4, 128).T)
    return (np.ascontiguousarray(w0), np.ascontiguousarray(w1), dwt, dbt)


def kernel(x, bw0, sw0, ss0, bw1, sw1, ss1, dw, db):
    if "nc" not in _cache:
        _cache["nc"] = _build_full()
    nc = _cache["nc"]
    w0, w1, dwt, dbt = _prep_weights(
        np.asarray(bw0, np.float32), np.asarray(sw0, np.float32),
        np.asarray(ss0, np.float32), np.asarray(bw1, np.float32),
        np.asarray(sw1, np.float32), np.asarray(ss1, np.float32),
        np.asarray(dw, np.float32), np.asarray(db, np.float32))
    xT = np.ascontiguousarray(np.asarray(x, np.float32).T)  # (1024, 4096)
    in_maps = []
    for c in range(NCORES):
        xc = np.ascontiguousarray(
            xT[:, c * BC:(c + 1) * BC].reshape(8, 128, BC))
        in_maps.append({"x_d": xc, "w0_d": w0, "w1_d": w1,
                        "dw_d": dwt, "db_d": dbt})
    _cache["in_maps"] = in_maps
    res = run_bass_kernel_spmd(nc, in_maps, list(range(NCORES)))
    out = np.empty((B, L), np.float32)
    for c in range(NCORES):
        oc = res.results[c]["o_d"]          # (4, 128, BC)
        out[c * BC:(c + 1) * BC, :] = oc.reshape(L, BC).T
    return out
